# revision 8
# baseline (speedup 1.0000x reference)
"""Trainium2 Bass kernel for nn_ODEG_8942121911067 (gnn_message_passing).

Math (derived from the reference ODE block; the Euler loop collapses to
its last step since f is recomputed from x_aug every iteration):

    out = relu(0.5*x_aug + 0.125*sigmoid(alpha)_i * (adj @ x_aug)
               + 0.25*S*R + 0.25*(x_aug @_t W2mix))

with x_aug = concat([x, zeros10], -1), S[b,n,t] = sum_f x_aug[b,n,t,f],
R[m] = sum_n ((w*clip(d,0,1)) @ w.T)[m,n], W2mix = (w2*clip(d2,0,1)) @ w2.T.

Device strategy (data-parallel over batch, 4 batches/core on 8 cores):
  - The node-mixing term runs as K=512 PSUM-accumulated matmuls on the
    PE with stationary A = diag(sigmoid(alpha)/8) @ adj (host-built).
    A and x_aug travel as fp8e4 with DoubleRow perf mode (K=256 per
    matmul). A is pre-scaled by a power of two into fp8 range (raw
    entries ~1e-4 would flush as subnormals); the descale rides the
    ACT eviction scale as a per-partition input. The adjacency term is
    ~1% of the output magnitude, so fp8 rounding there is negligible.
  - All precision-critical linear terms (0.5*x, the temporal T=24 mix,
    and the rank-1 S*R term over all 74 output columns - all
    layout-hostile to the PE but <5% of FLOPs) fold host-side into one
    bf16 side tensor q[..., 0:74], pre-scaled by the same power of two.
    A bf16 identity matmul accumulates q into the same PSUM bank as the
    adjacency chain, so the DVE does no work at all: the ACT engine
    evicts each [128, 24*74] PSUM block once with fused
    relu(psum * descale) -> bf16, and the output is upcast on the host.
  - The kernel is memory-bound: ~18.4 MB HBM traffic per core, with
    the PE and ACT each under the DMA time and overlapped.
"""

import numpy as np

B, N, T, F = 32, 512, 24, 64
NUM_ZEROS = 10
FA = F + NUM_ZEROS  # 74
N_CORES = 8
BPC = B // N_CORES  # batches per core = 4
NT = N // 128  # node chunks = 4
TFA = T * FA  # 1776 moving columns per node block
# PSUM-bank-aligned moving chunks: 512,512,512,240
CHUNKS = [(0, 512), (512, 512), (1024, 512), (1536, TFA - 1536)]

_CACHE = {}


def _build():
    import concourse.mybir as mybir
    import concourse.tile as tile
    from concourse import bacc

    bf16 = mybir.dt.bfloat16
    fp8 = mybir.dt.float8e4
    f32 = mybir.dt.float32

    nc = bacc.Bacc("TRN2", target_bir_lowering=False, debug=False,
                   num_devices=N_CORES)
    x_d = nc.dram_tensor("xin", [BPC, N, TFA], fp8, kind="ExternalInput").ap()
    q_d = nc.dram_tensor("q", [BPC, N, TFA], bf16, kind="ExternalInput").ap()
    at_d = nc.dram_tensor("at", [N, N], fp8, kind="ExternalInput").ap()
    id_d = nc.dram_tensor("idm", [128, 128], bf16, kind="ExternalInput").ap()
    sc_d = nc.dram_tensor("sc", [128, 1], f32, kind="ExternalInput").ap()
    out_d = nc.dram_tensor("out", [BPC, N, TFA], bf16, kind="ExternalOutput").ap()

    with tile.TileContext(nc) as tc:
        with (
            tc.tile_pool(name="const", bufs=1) as cpool,
            tc.tile_pool(name="xp", bufs=4) as xpool,
            tc.tile_pool(name="qp", bufs=4) as qpool,
            tc.tile_pool(name="op", bufs=8) as opool,
            tc.tile_pool(name="ps", bufs=2, space="PSUM") as pspool,
        ):
            atile = cpool.tile([128, NT, N], fp8, tag="at")
            nc.scalar.dma_start(
                atile[:], at_d[:].rearrange("(c p) n -> p c n", p=128))
            idt = cpool.tile([128, 128], bf16, tag="idm")
            nc.gpsimd.dma_start(idt[:], id_d[:])
            sc = cpool.tile([128, 1], f32, tag="sc")
            nc.gpsimd.dma_start(sc[:], sc_d[:])

            H = NT // 2
            for b in range(BPC):
                xv = x_d[b].rearrange("(h c p) tf -> h p c tf", h=2, p=128)
                qv = q_d[b].rearrange("(h c p) tf -> h p c tf", h=2, p=128)
                xhs = []
                qhs = []
                for h in range(2):
                    xh = xpool.tile([128, H, TFA], fp8, tag="xt")
                    xeng = nc.sync if (b + h) % 2 == 0 else nc.scalar
                    xeng.dma_start(xh[:], xv[h])
                    xhs.append(xh)
                    qh = qpool.tile([128, H, TFA], bf16, tag="qt")
                    qeng = nc.scalar if (b + h) % 2 == 0 else nc.sync
                    qeng.dma_start(qh[:], qv[h])
                    qhs.append(qh)
                for ic in range(NT):
                    qt = qhs[ic // H][:, ic % H]
                    ot = opool.tile([128, TFA], bf16, tag="ot")
                    ps = pspool.tile([128, TFA], f32, tag="ps")
                    for c0, cw in CHUNKS:
                        for h in range(2):
                            nc.tensor.matmul(
                                ps[:, c0:c0 + cw],
                                atile[:, 2 * h:2 * h + 2,
                                      ic * 128:(ic + 1) * 128],
                                xhs[h][:, :, c0:c0 + cw],
                                start=(h == 0),
                                stop=False,
                                perf_mode=mybir.MatmulPerfMode.DoubleRow,
                                skip_group_check=True,
                            )
                        nc.tensor.matmul(
                            ps[:, c0:c0 + cw],
                            idt[:],
                            qt[:, c0:c0 + cw],
                            start=False,
                            stop=True,
                            skip_group_check=True,
                        )
                    nc.scalar.activation(ot[:], ps[:],
                                         mybir.ActivationFunctionType.Relu,
                                         scale=sc[:, 0:1])
                    oeng = nc.scalar if ic % 2 == 0 else nc.sync
                    oeng.dma_start(out_d[b, ic * 128:(ic + 1) * 128], ot[:])

    nc.compile()
    return nc


def prepare(x, adj, alpha, w, d, w2, d2):
    """Host prep: fold parameters, build q. Returns (nc, in_maps)."""
    import ml_dtypes

    fp8 = ml_dtypes.float8_e4m3

    x = np.ascontiguousarray(np.asarray(x), np.float32)
    adj = np.asarray(adj)
    alpha = np.asarray(alpha)
    w = np.asarray(w)
    d = np.asarray(d)
    w2 = np.asarray(w2)
    d2 = np.asarray(d2)
    a = 1.0 / (1.0 + np.exp(-alpha.astype(np.float32)))
    A = 0.125 * a[:, None] * adj.astype(np.float32)

    # fp8e4 (e4m3, max 240): scale A and, if needed, x into range by
    # powers of two; the product of the inverses descales the PSUM.
    amax = max(float(np.abs(A).max()), 1e-30)
    sa = 2.0 ** np.floor(np.log2(120.0 / amax))
    xmax = max(float(np.abs(x).max()), 1e-30)
    sx = 2.0 ** min(np.floor(np.log2(120.0 / xmax)), 0.0)
    at = np.ascontiguousarray(A.T * sa, dtype=fp8)
    sq = np.float32(sa * sx)
    sc = np.full((128, 1), 1.0 / sq, np.float32)
    idm = np.eye(128, dtype=ml_dtypes.bfloat16)

    xs = x * sx if sx != 1.0 else x
    xa = np.zeros((B, N, T, FA), fp8)
    xa[..., :F] = xs.astype(fp8)

    dc = np.clip(d.astype(np.float32), 0.0, 1.0)
    W = (w.astype(np.float32) * dc) @ w.astype(np.float32).T
    R = W.sum(axis=1)  # [FA]
    d2c = np.clip(d2.astype(np.float32), 0.0, 1.0)
    W2 = (w2.astype(np.float32) * d2c) @ w2.astype(np.float32).T  # [T,T]

    S = x.sum(axis=3)  # [B,N,T]

    # q cols 0:64 = 0.5*x + 0.25*(x @_t W2) + 0.25*S*R[:64];
    # cols 64:74 = 0.25*S*R[64:74]; everything pre-scaled by sq so the
    # single ACT descale recovers the sum with the fp8 adjacency chain.
    q = np.empty((B, N, T, FA), np.float32)
    xt = np.matmul(x.transpose(0, 1, 3, 2), 0.25 * W2)  # [B,N,F,T]
    q[..., :F] = xt.transpose(0, 1, 3, 2)
    q[..., :F] += 0.5 * x
    q[...] += 0.25 * S[..., None] * R
    q *= sq
    q = q.astype(ml_dtypes.bfloat16)

    if "nc" not in _CACHE:
        _CACHE["nc"] = _build()
    nc = _CACHE["nc"]
    xa = xa.reshape(B, N, TFA)
    q = q.reshape(B, N, TFA)
    in_maps = [
        {"xin": xa[c * BPC:(c + 1) * BPC], "q": q[c * BPC:(c + 1) * BPC],
         "at": at, "idm": idm, "sc": sc}
        for c in range(N_CORES)
    ]
    return nc, in_maps


def kernel(x, adj, alpha, w, d, w2, d2):
    from concourse.bass_utils import run_bass_kernel_spmd

    nc, in_maps = prepare(x, adj, alpha, w, d, w2, d2)
    res = run_bass_kernel_spmd(nc, in_maps, list(range(N_CORES)))
    out = np.concatenate([res.results[c]["out"] for c in range(N_CORES)], axis=0)
    return np.ascontiguousarray(out, dtype=np.float32).reshape(B, N, T, FA)


# revision 12
# speedup vs baseline: 1.1819x; 1.1819x over previous
"""Trainium2 Bass kernel for nn_ODEG_8942121911067 (gnn_message_passing).

Math (derived from the reference ODE block; the Euler loop collapses to
its last step since f is recomputed from x_aug every iteration):

    out = relu(0.5*x_aug + 0.125*sigmoid(alpha)_i * (adj @ x_aug)
               + 0.25*S*R + 0.25*(x_aug @_t W2mix))

with x_aug = concat([x, zeros10], -1), S[b,n,t] = sum_f x_aug[b,n,t,f],
R[m] = sum_n ((w*clip(d,0,1)) @ w.T)[m,n], W2mix = (w2*clip(d2,0,1)) @ w2.T.

Device strategy (data-parallel over batch, 4 batches/core on 8 cores):
  - The node-mixing term runs as K=512 PSUM-accumulated matmuls on the
    PE with stationary A = diag(sigmoid(alpha)/8) @ adj (host-built).
    A and x travel as fp8e4 with DoubleRow perf mode (K=256/matmul).
    A is pre-scaled by a power of two into fp8 range (raw entries
    ~1e-4 would flush as subnormals); the descale rides the ACT
    eviction scale as a per-partition input. The adjacency term is ~1%
    of the output magnitude, so fp8 rounding there is negligible.
  - All precision-critical linear terms (0.5*x, the temporal T=24 mix,
    and the rank-1 S*R body term - all layout-hostile to the PE but <5%
    of FLOPs) fold host-side into one bf16 side tensor q[..., 0:64],
    pre-scaled by the same power of two; q[..., 64] carries S. A bf16
    identity matmul accumulates q into the same PSUM bank as the
    adjacency chain, so the ACT engine evicts each [128, 24*64] PSUM
    block once with fused relu(psum * descale) -> bf16. The 10
    zero-padding output columns are relu(0.25*S*R[64:74]), built on the
    otherwise-idle DVE as a stride-0-broadcast outer product + max.
  - All input tiles for the 4 local batches are prefetched up front on
    the gpsimd/vector queues (outputs ride scalar/sync), so the ~17 MB
    of HBM traffic per core streams continuously behind the PE.
"""

import numpy as np

B, N, T, F = 32, 512, 24, 64
NUM_ZEROS = 10
FA = F + NUM_ZEROS  # 74
FQ = F + 1  # q carries 64 real cols + one S column
N_CORES = 8
BPC = B // N_CORES  # batches per core = 4
NT = N // 128  # node chunks = 4
NCH = (T * F) // 512  # moving-dim chunks of 512 = 3
TPC = 512 // F  # t-values per 512-chunk = 8

_CACHE = {}


def _build():
    import concourse.mybir as mybir
    import concourse.tile as tile
    from concourse import bacc

    bf16 = mybir.dt.bfloat16
    fp8 = mybir.dt.float8e4
    f32 = mybir.dt.float32

    nc = bacc.Bacc("TRN2", target_bir_lowering=False, debug=False,
                   num_devices=N_CORES)
    x_d = nc.dram_tensor("xin", [BPC, N, T * F], fp8, kind="ExternalInput").ap()
    q_d = nc.dram_tensor("q", [BPC, N, T, FQ], bf16, kind="ExternalInput").ap()
    at_d = nc.dram_tensor("at", [N, N], fp8, kind="ExternalInput").ap()
    id_d = nc.dram_tensor("idm", [128, 128], bf16, kind="ExternalInput").ap()
    rp_d = nc.dram_tensor("rp", [128, NUM_ZEROS], f32, kind="ExternalInput").ap()
    sc_d = nc.dram_tensor("sc", [128, 1], f32, kind="ExternalInput").ap()
    out_d = nc.dram_tensor("out", [BPC, N, T, FA], bf16, kind="ExternalOutput").ap()

    with tile.TileContext(nc) as tc:
        with (
            tc.tile_pool(name="const", bufs=1) as cpool,
            tc.tile_pool(name="xp", bufs=2 * BPC) as xpool,
            tc.tile_pool(name="qp", bufs=2 * BPC) as qpool,
            tc.tile_pool(name="op", bufs=8) as opool,
            tc.tile_pool(name="ps", bufs=2, space="PSUM") as pspool,
        ):
            atile = cpool.tile([128, NT, N], fp8, tag="at")
            nc.gpsimd.dma_start(
                atile[:], at_d[:].rearrange("(c p) n -> p c n", p=128))
            idt = cpool.tile([128, 128], bf16, tag="idm")
            nc.gpsimd.dma_start(idt[:], id_d[:])
            sc = cpool.tile([128, 1], f32, tag="sc")
            nc.gpsimd.dma_start(sc[:], sc_d[:])
            rp = cpool.tile([128, 1, NUM_ZEROS], f32, tag="rp")
            nc.gpsimd.dma_start(rp[:], rp_d[:].rearrange("p (a b) -> p a b", a=1))

            H = NT // 2
            # Prefetch every batch's x and q tiles; pools are sized so
            # nothing recycles and all input DMA queues fill at t=0.
            xts = {}
            qts = {}
            for b in range(BPC):
                xv = x_d[b].rearrange("(h c p) tf -> h p c tf", h=2, p=128)
                qv = q_d[b].rearrange("(h c p) t f -> h p c t f", h=2, p=128)
                for h in range(2):
                    xh = xpool.tile([128, H, T * F], fp8, tag="xt")
                    nc.gpsimd.dma_start(xh[:], xv[h])
                    xts[b, h] = xh
                    qh = qpool.tile([128, H, T, FQ], bf16, tag="qt")
                    nc.sync.dma_start(qh[:], qv[h])
                    qts[b, h] = qh

            for b in range(BPC):
                for ic in range(NT):
                    qt = qts[b, ic // H][:, ic % H]
                    ot = opool.tile([128, T, FA], bf16, tag="ot")
                    ps = pspool.tile([128, T * F], f32, tag="ps")
                    for nch in range(NCH):
                        c0 = nch * 512
                        for h in range(2):
                            nc.tensor.matmul(
                                ps[:, c0:c0 + 512],
                                atile[:, 2 * h:2 * h + 2,
                                      ic * 128:(ic + 1) * 128],
                                xts[b, h][:, :, c0:c0 + 512],
                                start=(h == 0),
                                stop=False,
                                perf_mode=mybir.MatmulPerfMode.DoubleRow,
                                skip_group_check=True,
                            )
                        nc.tensor.matmul(
                            ps[:, c0:c0 + 512],
                            idt[:],
                            qt[:, nch * TPC:(nch + 1) * TPC, 0:F],
                            start=False,
                            stop=True,
                            skip_group_check=True,
                        )
                    nc.scalar.activation(
                        ot[:, :, 0:F],
                        ps[:].rearrange("p (t f) -> p t f", t=T),
                        mybir.ActivationFunctionType.Relu,
                        scale=sc[:, 0:1])
                    # pad cols: outer product S[p,t] * 0.25*R[f] then relu,
                    # both on the otherwise-idle DVE
                    nc.vector.scalar_tensor_tensor(
                        ot[:, :, F:FA],
                        qt[:, :, F:FQ].broadcast_to([128, T, NUM_ZEROS]),
                        1.0,
                        rp[:].broadcast_to([128, T, NUM_ZEROS]),
                        mybir.AluOpType.mult,
                        mybir.AluOpType.mult,
                    )
                    nc.vector.tensor_scalar_max(
                        ot[:, :, F:FA], ot[:, :, F:FA], 0.0)
                    oeng = nc.gpsimd if ic % 2 == 0 else nc.sync
                    oeng.dma_start(out_d[b, ic * 128:(ic + 1) * 128], ot[:])

    nc.compile()
    return nc


def prepare(x, adj, alpha, w, d, w2, d2):
    """Host prep: fold parameters, build q. Returns (nc, in_maps)."""
    import ml_dtypes

    fp8 = ml_dtypes.float8_e4m3

    x = np.ascontiguousarray(np.asarray(x), np.float32)
    adj = np.asarray(adj)
    alpha = np.asarray(alpha)
    w = np.asarray(w)
    d = np.asarray(d)
    w2 = np.asarray(w2)
    d2 = np.asarray(d2)
    a = 1.0 / (1.0 + np.exp(-alpha.astype(np.float32)))
    A = 0.125 * a[:, None] * adj.astype(np.float32)

    # fp8e4 (e4m3, max 240): scale A and, if needed, x into range by
    # powers of two; the product of the inverses descales the PSUM.
    amax = max(float(np.abs(A).max()), 1e-30)
    sa = 2.0 ** np.floor(np.log2(120.0 / amax))
    xmax = max(float(np.abs(x).max()), 1e-30)
    sx = 2.0 ** min(np.floor(np.log2(120.0 / xmax)), 0.0)
    at = np.ascontiguousarray(A.T * sa, dtype=fp8)
    xb = ((x * sx) if sx != 1.0 else x).astype(fp8)
    sq = np.float32(sa * sx)
    sc = np.full((128, 1), 1.0 / sq, np.float32)
    idm = np.eye(128, dtype=ml_dtypes.bfloat16)

    dc = np.clip(d.astype(np.float32), 0.0, 1.0)
    W = (w.astype(np.float32) * dc) @ w.astype(np.float32).T
    R = W.sum(axis=1)  # [FA]
    d2c = np.clip(d2.astype(np.float32), 0.0, 1.0)
    W2 = (w2.astype(np.float32) * d2c) @ w2.astype(np.float32).T  # [T,T]

    S = x.sum(axis=3)  # [B,N,T]
    rp = np.ascontiguousarray(
        np.broadcast_to(0.25 * R[F:], (128, NUM_ZEROS)), np.float32)

    # q cols 0:64 = sq*(0.5*x + 0.25*(x @_t W2) + 0.25*S*R[:64]) so the
    # single ACT descale recovers the sum with the fp8 adjacency chain;
    # col 64 = S (unscaled, feeds the DVE pad outer product).
    q = np.empty((B, N, T, FQ), np.float32)
    xt = np.matmul(x.transpose(0, 1, 3, 2), 0.25 * W2)  # [B,N,F,T]
    q[..., :F] = xt.transpose(0, 1, 3, 2)
    q[..., :F] += 0.5 * x
    q[..., :F] += 0.25 * S[..., None] * R[:F]
    q[..., :F] *= sq
    q[..., F] = S
    q = q.astype(ml_dtypes.bfloat16)

    if "nc" not in _CACHE:
        _CACHE["nc"] = _build()
    nc = _CACHE["nc"]
    xb = xb.reshape(B, N, T * F)
    in_maps = [
        {"xin": xb[c * BPC:(c + 1) * BPC], "q": q[c * BPC:(c + 1) * BPC],
         "at": at, "idm": idm, "rp": rp, "sc": sc}
        for c in range(N_CORES)
    ]
    return nc, in_maps


def kernel(x, adj, alpha, w, d, w2, d2):
    from concourse.bass_utils import run_bass_kernel_spmd

    nc, in_maps = prepare(x, adj, alpha, w, d, w2, d2)
    res = run_bass_kernel_spmd(nc, in_maps, list(range(N_CORES)))
    out = np.concatenate([res.results[c]["out"] for c in range(N_CORES)], axis=0)
    return np.ascontiguousarray(out, dtype=np.float32)


# revision 13
# speedup vs baseline: 1.2472x; 1.0553x over previous
"""Trainium2 Bass kernel for nn_ODEG_8942121911067 (gnn_message_passing).

Math (derived from the reference ODE block; the Euler loop collapses to
its last step since f is recomputed from x_aug every iteration):

    out = relu(0.5*x_aug + 0.125*sigmoid(alpha)_i * (adj @ x_aug)
               + 0.25*S*R + 0.25*(x_aug @_t W2mix))

with x_aug = concat([x, zeros10], -1), S[b,n,t] = sum_f x_aug[b,n,t,f],
R[m] = sum_n ((w*clip(d,0,1)) @ w.T)[m,n], W2mix = (w2*clip(d2,0,1)) @ w2.T.

The 10 zero-padding output columns are relu(0.25*S*R[64:74]) - a rank-1
outer product with no adjacency/temporal coupling - and are filled on
the host; the device computes the 64 real columns.

Device strategy (data-parallel over batch, 4 batches/core on 8 cores):
  - The node-mixing term runs as K=512 PSUM-accumulated matmuls on the
    PE with stationary A = diag(sigmoid(alpha)/8) @ adj (host-built).
    A and x travel as fp8e4 with DoubleRow perf mode (K=256/matmul).
    A is pre-scaled by a power of two into fp8 range (raw entries
    ~1e-4 would flush as subnormals); the descale rides the ACT
    eviction scale as a per-partition input. The adjacency term is ~1%
    of the output magnitude, so fp8 rounding there is negligible.
  - All precision-critical linear terms (0.5*x, the temporal T=24 mix,
    and the rank-1 S*R body term - all layout-hostile to the PE but <5%
    of FLOPs) fold host-side into one bf16 side tensor q, pre-scaled by
    the same power of two. A bf16 identity matmul accumulates q into
    the same PSUM bank as the adjacency chain, so the ACT engine evicts
    each [128, 24*64] PSUM block once with relu(psum * descale) -> bf16.
  - The kernel is at the HBM roofline (~16 MB/core): inputs prefetch
    up front on the gpsimd+scalar DMA rings (batch-interleaved), the
    sync ring carries only outputs so evictions never queue behind
    prefetches, and the PE/ACT both fit under the DMA time.
"""

import numpy as np

B, N, T, F = 32, 512, 24, 64
NUM_ZEROS = 10
FA = F + NUM_ZEROS  # 74
N_CORES = 8
BPC = B // N_CORES  # batches per core = 4
NT = N // 128  # node chunks = 4
NCH = (T * F) // 512  # moving-dim chunks of 512 = 3
TPC = 512 // F  # t-values per 512-chunk = 8

_CACHE = {}


def _build():
    import concourse.mybir as mybir
    import concourse.tile as tile
    from concourse import bacc

    bf16 = mybir.dt.bfloat16
    fp8 = mybir.dt.float8e4
    f32 = mybir.dt.float32

    nc = bacc.Bacc("TRN2", target_bir_lowering=False, debug=False,
                   num_devices=N_CORES)
    x_d = nc.dram_tensor("xin", [BPC, N, T * F], fp8, kind="ExternalInput").ap()
    q_d = nc.dram_tensor("q", [BPC, N, T * F], bf16, kind="ExternalInput").ap()
    at_d = nc.dram_tensor("at", [N, N], fp8, kind="ExternalInput").ap()
    id_d = nc.dram_tensor("idm", [128, 128], bf16, kind="ExternalInput").ap()
    sc_d = nc.dram_tensor("sc", [128, 1], f32, kind="ExternalInput").ap()
    out_d = nc.dram_tensor("out", [BPC, N, T * F], bf16, kind="ExternalOutput").ap()

    with tile.TileContext(nc) as tc:
        with (
            tc.tile_pool(name="const", bufs=1) as cpool,
            tc.tile_pool(name="xp", bufs=2 * BPC) as xpool,
            tc.tile_pool(name="qp", bufs=2 * BPC) as qpool,
            tc.tile_pool(name="op", bufs=8) as opool,
            tc.tile_pool(name="ps", bufs=2, space="PSUM") as pspool,
        ):
            atile = cpool.tile([128, NT, N], fp8, tag="at")
            nc.gpsimd.dma_start(
                atile[:], at_d[:].rearrange("(c p) n -> p c n", p=128))
            idt = cpool.tile([128, 128], bf16, tag="idm")
            nc.gpsimd.dma_start(idt[:], id_d[:])
            sc = cpool.tile([128, 1], f32, tag="sc")
            nc.gpsimd.dma_start(sc[:], sc_d[:])

            H = NT // 2
            # Prefetch every batch's x and q tiles, batch-interleaved so
            # early blocks' inputs land first. The sync ring is reserved
            # for outputs so evictions never queue behind these.
            xts = {}
            qts = {}
            for b in range(BPC):
                xv = x_d[b].rearrange("(h c p) tf -> h p c tf", h=2, p=128)
                qv = q_d[b].rearrange("(h c p) tf -> h p c tf", h=2, p=128)
                for h in range(2):
                    xh = xpool.tile([128, H, T * F], fp8, tag="xt")
                    xeng = nc.scalar if h == 0 else nc.gpsimd
                    xeng.dma_start(xh[:], xv[h])
                    xts[b, h] = xh
                    qh = qpool.tile([128, H, T * F], bf16, tag="qt")
                    qeng = nc.gpsimd if h == 0 else nc.scalar
                    qeng.dma_start(qh[:], qv[h])
                    qts[b, h] = qh

            for b in range(BPC):
                for ic in range(NT):
                    qt = qts[b, ic // H][:, ic % H]
                    ot = opool.tile([128, T * F], bf16, tag="ot")
                    ps = pspool.tile([128, T * F], f32, tag="ps")
                    for nch in range(NCH):
                        c0 = nch * 512
                        for h in range(2):
                            nc.tensor.matmul(
                                ps[:, c0:c0 + 512],
                                atile[:, 2 * h:2 * h + 2,
                                      ic * 128:(ic + 1) * 128],
                                xts[b, h][:, :, c0:c0 + 512],
                                start=(h == 0),
                                stop=False,
                                perf_mode=mybir.MatmulPerfMode.DoubleRow,
                                skip_group_check=True,
                            )
                        nc.tensor.matmul(
                            ps[:, c0:c0 + 512],
                            idt[:],
                            qt[:, c0:c0 + 512],
                            start=False,
                            stop=True,
                            skip_group_check=True,
                        )
                    nc.scalar.activation(
                        ot[:], ps[:],
                        mybir.ActivationFunctionType.Relu,
                        scale=sc[:, 0:1])
                    nc.sync.dma_start(out_d[b, ic * 128:(ic + 1) * 128], ot[:])

    nc.compile()
    return nc


def prepare(x, adj, alpha, w, d, w2, d2):
    """Host prep: fold parameters, build q. Returns (nc, in_maps)."""
    import ml_dtypes

    fp8 = ml_dtypes.float8_e4m3

    x = np.ascontiguousarray(np.asarray(x), np.float32)
    adj = np.asarray(adj)
    alpha = np.asarray(alpha)
    w = np.asarray(w)
    d = np.asarray(d)
    w2 = np.asarray(w2)
    d2 = np.asarray(d2)
    a = 1.0 / (1.0 + np.exp(-alpha.astype(np.float32)))
    A = 0.125 * a[:, None] * adj.astype(np.float32)

    # fp8e4 (e4m3, max 240): scale A and, if needed, x into range by
    # powers of two; the product of the inverses descales the PSUM.
    amax = max(float(np.abs(A).max()), 1e-30)
    sa = 2.0 ** np.floor(np.log2(120.0 / amax))
    xmax = max(float(np.abs(x).max()), 1e-30)
    sx = 2.0 ** min(np.floor(np.log2(120.0 / xmax)), 0.0)
    at = np.ascontiguousarray(A.T * sa, dtype=fp8)
    xb = ((x * sx) if sx != 1.0 else x).astype(fp8)
    sq = np.float32(sa * sx)
    sc = np.full((128, 1), 1.0 / sq, np.float32)
    idm = np.eye(128, dtype=ml_dtypes.bfloat16)

    dc = np.clip(d.astype(np.float32), 0.0, 1.0)
    W = (w.astype(np.float32) * dc) @ w.astype(np.float32).T
    R = W.sum(axis=1)  # [FA]
    d2c = np.clip(d2.astype(np.float32), 0.0, 1.0)
    W2 = (w2.astype(np.float32) * d2c) @ w2.astype(np.float32).T  # [T,T]

    S = x.sum(axis=3)  # [B,N,T]

    # q = sq*(0.5*x + 0.25*(x @_t W2) + 0.25*S*R[:64]); the single ACT
    # descale recovers the sum with the fp8 adjacency chain.
    xt = np.matmul(x.transpose(0, 1, 3, 2), 0.25 * W2)  # [B,N,F,T]
    q = np.ascontiguousarray(xt.transpose(0, 1, 3, 2))
    q += 0.5 * x
    q += 0.25 * S[..., None] * R[:F]
    q *= sq
    q = q.astype(ml_dtypes.bfloat16)

    # Host-side pad columns: relu(0.25 * S * R[64:74])
    pads = np.maximum(0.25 * S[..., None] * R[F:], 0.0).astype(np.float32)
    _CACHE["pads"] = pads

    if "nc" not in _CACHE:
        _CACHE["nc"] = _build()
    nc = _CACHE["nc"]
    xb = xb.reshape(B, N, T * F)
    q = q.reshape(B, N, T * F)
    in_maps = [
        {"xin": xb[c * BPC:(c + 1) * BPC], "q": q[c * BPC:(c + 1) * BPC],
         "at": at, "idm": idm, "sc": sc}
        for c in range(N_CORES)
    ]
    return nc, in_maps


def assemble(results):
    """Concatenate per-core outputs, upcast, and add host pad columns."""
    dev = np.concatenate([results[c]["out"] for c in range(N_CORES)], axis=0)
    out = np.empty((B, N, T, FA), np.float32)
    out[..., :F] = dev.reshape(B, N, T, F).astype(np.float32)
    out[..., F:] = _CACHE["pads"]
    return out


def kernel(x, adj, alpha, w, d, w2, d2):
    from concourse.bass_utils import run_bass_kernel_spmd

    nc, in_maps = prepare(x, adj, alpha, w, d, w2, d2)
    res = run_bass_kernel_spmd(nc, in_maps, list(range(N_CORES)))
    return assemble(res.results)
